# revision 46
# baseline (speedup 1.0000x reference)
"""Memory-efficient multi-head attention on 8 TRN2 NeuronCores (bf16 core).

Problem (hardcoded): B=2, L=2048, D=1024, H=16, HD=64.
  out = softmax((X_q Wq + bq)(X_k Wk + bk)^T / sqrt(HD)) (X_v Wv + bv) Wo + bo

Sharding: 8 cores = 2 batches x 4 head-groups (4 heads each).  Each core gets
its batch's activations (transposed, bf16) plus its head-group's weight
slices, and produces a partial pre-bias output outT [D, L] in fp32.  Host
sums the 4 partials per batch and adds bo.

Device kernel design (cost-model-driven):
  - All matmul operands are bf16 (1 PE cycle/row, same as fp32r, but half
    the HBM traffic and SBUF).  PSUM accumulation stays fp32.  bf16
    per-element rounding (~0.1% rms) contributes ~0.2% output error vs the
    2e-2 budget.  (fp8 DoubleRow was evaluated and measured ~9% error - the
    attention average shrinks signal as fast as noise, so per-element fp8
    error passes through at full strength.  Rejected.)
  - Scores are computed transposed, sT[kpos, q] = k . q, per head on disjoint
    64-partition groups; softmax denominator comes from an appended ones
    column on v (attnv accumulator row 64).
  - exp on ScalarE with the 1/sqrt(HD) scale folded in, no max-subtraction
    (scores ~N(0,1); softmax is shift-invariant; fp32/bf16 range is ample).
  - Biases are applied by DVE during PSUM->SBUF copy-out (tensor_scalar_add
    with per-partition bias for q/k, tensor_tensor add with a host-
    broadcast bias tile for v) - no PE bias matmuls.
  - A Pool-engine memset chain paces tiny dummy matmuls over the first
    ~6us so the PE p-state ramp (cost model: warm after 3us of continuous
    activity; resets only after a >3us idle gap) is done before real
    matmuls arrive - every real matmul runs at the warm 2.4GHz rate.
  - PE is the bottleneck (163.7us busy, its exact cost-model floor).
    PSUM banks: stA/stB [128,1024] score slots (the two heads' exps
    alternate, so each head's next score lands while the other head's exp
    runs - zero-stall ACT cadence) + ot0/ot1 [65,1024] attnv accumulators.
    All projection and out-proj(qc0) "filler" work runs through the ot
    slot ring: before each pair's first attnv (deferred via the 10-deep
    at-tile pool) and at pair boundaries, with deadlines matched to the
    single-queue DMA arrival order so nothing parks in the 4-deep PE wait
    queue.  out-proj(qc1) is the tail, cs-ordered behind the chunked
    final epilogue, with copies split between the then-idle ACT and DVE
    and bf16 stores to halve the final DMA.
"""

import os
import numpy as np
import ml_dtypes

EMIT_LOG = []  # (inst_name, label) when KERNEL_EMIT_LOG=1

import concourse.mybir as mybir
import concourse.tile as tile
from concourse import bacc
from concourse.bass_utils import run_bass_kernel_spmd

F32 = mybir.dt.float32
BF16 = mybir.dt.bfloat16
NPBF16 = ml_dtypes.bfloat16


def build_attention_core(L=2048, D=1024, H_LOC=4, HD=64):
    JC = H_LOC * HD                   # local head channels (256)
    NJT = JC // 128                   # j-tiles (2)
    NDT = D // 128                    # contraction tiles (8)
    NLT = L // 128                    # k-position tiles (16)
    XC = 512                          # x stream chunk width
    NXC = L // XC                     # 4
    QC = 1024                         # attention q-chunk (exp tile width)
    NQC = L // QC                     # 2
    CS = 512                          # matmul N-chunk inside a q-chunk
    NCS = QC // CS                    # 2

    nc = bacc.Bacc("TRN2", target_bir_lowering=False, debug=False, num_devices=8)

    xq = nc.dram_tensor("xq", [128, NDT, L], BF16, kind="ExternalInput").ap()
    xk = nc.dram_tensor("xk", [128, NDT, L], BF16, kind="ExternalInput").ap()
    xv = nc.dram_tensor("xv", [128, NDT, L], BF16, kind="ExternalInput").ap()
    wq = nc.dram_tensor("wq", [128, NDT, JC], BF16, kind="ExternalInput").ap()
    wk = nc.dram_tensor("wk", [128, NDT, JC], BF16, kind="ExternalInput").ap()
    wv = nc.dram_tensor("wv", [128, NDT, JC], BF16, kind="ExternalInput").ap()
    wo = nc.dram_tensor("wo", [128, NJT, D], BF16, kind="ExternalInput").ap()
    bq = nc.dram_tensor("bq", [128, NJT], F32, kind="ExternalInput").ap()
    bk = nc.dram_tensor("bk", [128, NJT], F32, kind="ExternalInput").ap()
    bv = nc.dram_tensor("bv", [128, JC], F32, kind="ExternalInput").ap()
    outT = nc.dram_tensor("outT", [D, L], BF16, kind="ExternalOutput").ap()

    from contextlib import ExitStack

    with (
        tile.TileContext(nc) as tc,
        ExitStack() as stack,
        nc.allow_low_precision(reason="bf16 operands, fp32 accumulation"),
    ):
        consts = stack.enter_context(tc.tile_pool(name="consts", bufs=1))
        warm = stack.enter_context(tc.tile_pool(name="warm", bufs=1))
        wpool = stack.enter_context(tc.tile_pool(name="wpool", bufs=1))
        xpool = stack.enter_context(tc.tile_pool(name="xpool", bufs=1))
        prod = stack.enter_context(tc.tile_pool(name="prod", bufs=1))
        att_pool = stack.enter_context(tc.tile_pool(name="att", bufs=10))
        rden_pool = stack.enter_context(tc.tile_pool(name="rden", bufs=2))
        oc_pool = stack.enter_context(tc.tile_pool(name="oc", bufs=8))
        psum = stack.enter_context(tc.tile_pool(name="psum", bufs=1, space="PSUM"))

        # ---- PE warmup: Pool memset chain paces tiny matmuls ~1.5us apart
        # (prevents a pe_busy_start reset during the DMA prefix; each reset
        # makes the next ~3us of matmuls run at the cold p-state)
        wsrc = warm.tile([1, 8], BF16, tag="wsrc")
        nc.vector.memset(wsrc, 0.0)
        ps_wm = psum.tile([1, 8], F32, tag="stA", name="ps_wm")
        for i in range(2):
            wch = warm.tile([1, 1500], F32, tag="wch", name="wch")
            nc.gpsimd.memset(wch, 0.0)
            gate = warm.tile([1, 2], BF16, tag="wg", bufs=2, name="gate")
            nc.vector.tensor_copy(out=gate, in_=wch[:, 0:2])
            nc.tensor.matmul(ps_wm[:, 0:1], gate[:, 0:1], gate[:, 1:2],
                             start=True, stop=True)

        # ---- weights + biases (lead the DMA queue)
        wk_sb = wpool.tile([128, NDT, JC], BF16, tag="wk")
        wq_sb = wpool.tile([128, NDT, JC], BF16, tag="wq")
        wv_sb = wpool.tile([128, NDT, JC], BF16, tag="wv")
        wo_sb = wpool.tile([128, NJT, D], BF16, tag="wo")
        bq_sb = wpool.tile([128, NJT], F32, tag="bq")
        bk_sb = wpool.tile([128, NJT], F32, tag="bk")
        bv_sb = wpool.tile([128, JC], F32, tag="bv")
        nc.sync.dma_start(out=wk_sb[:, 0:4, :], in_=wk[:, 0:4, :])

        # ---- x streams (SBUF-resident, chunked DMA)
        xk_sb = xpool.tile([128, NDT, L], BF16, tag="xk")
        xq_sb = xpool.tile([128, NDT, L], BF16, tag="xq")
        xv_sb = xpool.tile([128, NDT, L], BF16, tag="xv")

        def load_x(eng, x_sb, x_dram, c):
            eng.dma_start(out=x_sb[:, :, c * XC:(c + 1) * XC],
                          in_=x_dram[:, :, c * XC:(c + 1) * XC])

        # All loads on ONE queue (SP) in priority order: the DMA_ENGINES
        # device serializes transfers, so a second queue would let late
        # loads jump ahead of the critical prefix.
        nc.sync.dma_start(out=xk_sb[:, 0:4, 0:XC], in_=xk[:, 0:4, 0:XC])
        nc.sync.dma_start(out=bk_sb, in_=bk)
        nc.sync.dma_start(out=wk_sb[:, 4:8, :], in_=wk[:, 4:8, :])
        nc.sync.dma_start(out=xk_sb[:, 4:8, 0:XC], in_=xk[:, 4:8, 0:XC])
        nc.sync.dma_start(out=wq_sb[:, 0:4, :], in_=wq[:, 0:4, :])
        nc.sync.dma_start(out=xq_sb[:, 0:4, 0:XC], in_=xq[:, 0:4, 0:XC])
        nc.sync.dma_start(out=bq_sb, in_=bq)
        nc.sync.dma_start(out=wq_sb[:, 4:8, :], in_=wq[:, 4:8, :])
        nc.sync.dma_start(out=xq_sb[:, 4:8, 0:XC], in_=xq[:, 4:8, 0:XC])
        load_x(nc.sync, xq_sb, xq, 1)
        nc.sync.dma_start(out=wv_sb, in_=wv)
        nc.sync.dma_start(out=bv_sb, in_=bv)
        load_x(nc.sync, xk_sb, xk, 1)
        load_x(nc.sync, xv_sb, xv, 0)
        load_x(nc.sync, xk_sb, xk, 2)
        load_x(nc.sync, xv_sb, xv, 1)
        load_x(nc.sync, xk_sb, xk, 3)
        load_x(nc.sync, xv_sb, xv, 2)
        load_x(nc.sync, xv_sb, xv, 3)
        load_x(nc.sync, xq_sb, xq, 2)
        load_x(nc.sync, xq_sb, xq, 3)
        nc.sync.dma_start(out=wo_sb, in_=wo)

        # ---- products
        qT_sb = prod.tile([128, NJT, L], BF16, tag="qT")    # [j%128, jt, l]
        kT_sb = prod.tile([128, NJT, L], BF16, tag="kT")
        v_sb = prod.tile([128, NLT, H_LOC, HD + 1], BF16, tag="v")
        onorm_sb = prod.tile([128, NJT, L], BF16, tag="onorm")

        vones_f32 = consts.tile([128, NLT * H_LOC], F32)
        nc.vector.memset(vones_f32, 1.0)
        nc.vector.tensor_copy(
            out=v_sb[:, :, :, HD:HD + 1].rearrange("p a b c -> p (a b c)"),
            in_=vones_f32,
        )

        # ---- emission helpers ------------------------------------------
        # Projections are emitted as small "pieces" (2 contraction matmuls
        # each, ~430ns of PE) so that, interleaved into the attention
        # stream, they never delay the next score matmul in the in-order
        # PE queue by more than ~one piece.
        def proj_kq_pieces(w_sb, b_sb, dst, c, jt, tag="fl"):
            x_sb = xk_sb if dst is kT_sb else xq_sb
            state = {}

            def piece(p):
                def emit():
                    if p == 0:
                        state["ps"] = psum.tile([128, XC], F32, tag=tag,
                                                name="pskq")
                    _log(f"projkq c{c} jt{jt} p{p}")
                    ps = state["ps"]
                    for dt in (2 * p, 2 * p + 1):
                        nc.tensor.matmul(
                            ps,
                            w_sb[:, dt, jt * 128:(jt + 1) * 128],
                            x_sb[:, dt, c * XC:(c + 1) * XC],
                            start=(dt == 0),
                            stop=(dt == NDT - 1),
                        )
                    if p == NDT // 2 - 1:
                        nc.vector.tensor_scalar_add(
                            out=dst[:, jt, c * XC:(c + 1) * XC],
                            in0=ps,
                            scalar1=b_sb[:, jt:jt + 1],
                        )
                return emit
            return [piece(p) for p in range(NDT // 2)]

        def proj_v_pieces(lt, tag="fl"):
            state = {}

            def piece(p):
                def emit():
                    if p == 0:
                        state["ps"] = psum.tile([128, JC], F32, tag=tag,
                                                name="psv")
                    _log(f"projv lt{lt} p{p}")
                    ps = state["ps"]
                    for dt in (2 * p, 2 * p + 1):
                        nc.tensor.matmul(
                            ps,
                            xv_sb[:, dt, lt * 128:lt * 128 + 128],
                            wv_sb[:, dt, :],
                            start=(dt == 0),
                            stop=(dt == NDT - 1),
                        )
                    if p == NDT // 2 - 1:
                        nc.vector.tensor_tensor(
                            out=v_sb[:, lt, :, 0:HD],
                            in0=ps.rearrange("p (h d) -> p h d", h=H_LOC),
                            in1=bv_sb.rearrange("p (h d) -> p h d", h=H_LOC),
                            op=mybir.AluOpType.add,
                        )
                return emit
            return [piece(p) for p in range(NDT // 2)]

        EXP_SCALE = float(1.0 / np.sqrt(HD))

        _log_on = os.environ.get("KERNEL_EMIT_LOG") == "1"

        def _log(label):
            if _log_on:
                # peek: next add_instruction gets this id + 1 (next_id consumes one)
                nid = nc.next_id()
                EMIT_LOG.append((f"I-{nid + 1}", label))

        def _score_mm(ps, hb, hp, kt, q0, q1):
            _log(f"score hb{hb} hp{hp} kt{kt} q{q0}")
            nc.tensor.matmul(
                ps,
                kT_sb[hb:hb + HD, hp, kt * 128:kt * 128 + 128],
                qT_sb[hb:hb + HD, hp, q0:q1],
                start=True,
                stop=True,
            )

        def _exp(out, in_):
            _log("exp")
            nc.scalar.activation(out=out, in_=in_,
                                 func=mybir.ActivationFunctionType.Exp,
                                 scale=EXP_SCALE)

        def sc_h(hp, qc, kt, h, tag, split0=False):
            """One head's scores for a kt into its dedicated 2-bank slot."""
            q0 = qc * QC
            hb = h * HD
            if split0:
                tiles = []
                for c in range(NCS):
                    st = psum.tile([128, CS], F32, tag=tag, name=tag)
                    _score_mm(st, hb, hp, kt, q0 + c * CS, q0 + (c + 1) * CS)
                    tiles.append(st)
                return tiles
            st = psum.tile([128, QC], F32, tag=tag, name=tag)
            for c in range(NCS):
                _score_mm(st[:, c * CS:(c + 1) * CS], hb, hp, kt,
                          q0 + c * CS, q0 + (c + 1) * CS)
            return st

        def attnv(hp, kt, h, at, ot):
            hg = 2 * hp + (h % 2)
            for c in range(NCS):
                _log(f"attnv hp{hp} kt{kt} h{h}")
                nc.tensor.matmul(
                    ot[:, c * CS:(c + 1) * CS],
                    v_sb[:, kt, hg, :],
                    at[:, c * CS:(c + 1) * CS],
                    start=(kt == 0),
                    stop=(kt == NLT - 1),
                )

        def epilogue(hp, qc, ots, chunks=1):
            """onorm[ch, q] = OT[ch, q] * recip(OT[HD, q])."""
            W = QC // chunks
            for cc in range(chunks):
                for h in (0, 1):
                    hb = h * HD
                    rden = rden_pool.tile([1, W], F32, tag="rden", name="rden")
                    nc.vector.reciprocal(
                        out=rden, in_=ots[h][HD:HD + 1, cc * W:(cc + 1) * W])
                    rbc = rden_pool.tile([HD, W], F32, tag="rbc", name="rbc")
                    nc.gpsimd.partition_broadcast(rbc, rden)
                    nc.vector.tensor_mul(
                        out=onorm_sb[hb:hb + HD, hp,
                                     qc * QC + cc * W:qc * QC + (cc + 1) * W],
                        in0=ots[h][0:HD, cc * W:(cc + 1) * W],
                        in1=rbc,
                    )

        _store_rr = [0]

        def outproj_unit(qc, mt, c, tag, copy_eng):
            """One [128, CS] tile of outT[dp, l] = sum_j wo[j, dp] onorm[j, l]."""
            ps = psum.tile([128, CS], F32, tag=tag, name="psc")
            _log(f"outproj qc{qc} mt{mt} c{c}")
            for jt in range(NJT):
                nc.tensor.matmul(
                    ps,
                    wo_sb[:, jt, mt * 128:(mt + 1) * 128],
                    onorm_sb[:, jt, qc * QC + c * CS:qc * QC + (c + 1) * CS],
                    start=(jt == 0),
                    stop=(jt == NJT - 1),
                )
            ob = oc_pool.tile([128, CS], BF16, tag="oc", name="oc")
            if copy_eng is nc.scalar:
                nc.scalar.copy(out=ob, in_=ps)
            else:
                copy_eng.tensor_copy(out=ob, in_=ps)
            dma_eng = nc.sync  # SP queue is idle once loads finish
            dma_eng.dma_start(
                out=outT[mt * 128:(mt + 1) * 128,
                         qc * QC + c * CS:qc * QC + (c + 1) * CS],
                in_=ob,
            )

        # ---- emission schedule -----------------------------------------
        # Both heads' exps are [128, QC] through dedicated 2-bank slots
        # (stA/stB) - the zero-stall cadence: while exp(h0) runs, PE scores
        # h1 into stB, and vice versa.  attnv trails via the deep at pool
        # (12 bufs/head), so ALL projection/out-proj filler units run in
        # the ot-slot ring: before the pair's first attnv (window 0 covers
        # every projection with an early deadline) and at pair boundaries
        # after each epilogue.  ACT stalls where PE is the binding engine;
        # PE should never idle.
        for em in proj_kq_pieces(wk_sb, bk_sb, kT_sb, 0, 0, tag="stA"):
            em()
        for em in proj_kq_pieces(wq_sb, bq_sb, qT_sb, 0, 0, tag="stB"):
            em()

        # filler units, deadline = global step by which the unit must be
        # DONE; emitted into the ot ring at window points.  DMA arrivals
        # (one load queue) gate the earliest useful emission.
        units = []
        units.append((1, proj_kq_pieces(wq_sb, bq_sb, qT_sb, 1, 0, tag="ot0")))
        units.append((3, proj_kq_pieces(wk_sb, bk_sb, kT_sb, 1, 0, tag="ot0")))
        units.append((2, proj_v_pieces(0, tag="ot1")))
        units.append((2, proj_v_pieces(1, tag="ot0")))
        units.append((3, proj_v_pieces(2, tag="ot1")))
        units.append((4, proj_v_pieces(3, tag="ot0")))
        units.append((5, proj_v_pieces(4, tag="ot1")))
        units.append((5, proj_kq_pieces(wk_sb, bk_sb, kT_sb, 2, 0, tag="ot0")))
        units.append((6, proj_v_pieces(5, tag="ot1")))
        units.append((0, proj_kq_pieces(wk_sb, bk_sb, kT_sb, 0, 1, tag="ot0")))
        units.append((7, proj_v_pieces(6, tag="ot1")))
        units.append((7, proj_kq_pieces(wq_sb, bq_sb, qT_sb, 0, 1, tag="ot0")))
        units.append((8, proj_v_pieces(7, tag="ot1")))
        units.append((8, proj_kq_pieces(wk_sb, bk_sb, kT_sb, 3, 0, tag="ot0")))
        units.append((9, proj_v_pieces(8, tag="ot1")))
        units.append((9, proj_kq_pieces(wq_sb, bq_sb, qT_sb, 1, 1, tag="ot0")))
        units.append((10, proj_v_pieces(9, tag="ot1")))
        units.append((10, proj_kq_pieces(wk_sb, bk_sb, kT_sb, 1, 1, tag="ot0")))
        units.append((11, proj_v_pieces(10, tag="ot1")))
        units.append((11, proj_v_pieces(11, tag="ot0")))
        units.append((12, proj_v_pieces(12, tag="ot1")))
        units.append((12, proj_kq_pieces(wk_sb, bk_sb, kT_sb, 2, 1, tag="ot0")))
        units.append((13, proj_v_pieces(13, tag="ot1")))
        units.append((13, proj_kq_pieces(wk_sb, bk_sb, kT_sb, 3, 1, tag="ot0")))
        units.append((14, proj_v_pieces(14, tag="ot1")))
        units.append((14, proj_v_pieces(15, tag="ot1")))
        units.append((18, proj_kq_pieces(wq_sb, bq_sb, qT_sb, 2, 0, tag="ot0")))
        units.append((19, proj_kq_pieces(wq_sb, bq_sb, qT_sb, 3, 0, tag="ot1")))
        units.append((33, proj_kq_pieces(wq_sb, bq_sb, qT_sb, 2, 1, tag="ot0")))
        units.append((34, proj_kq_pieces(wq_sb, bq_sb, qT_sb, 3, 1, tag="ot1")))
        for mt in range(NDT - 3):
            for cs in range(NCS):
                units.append((35 + (2 * mt + cs) % 10,
                              [lambda mt=mt, cs=cs, tg=f"ot{(2*mt+cs) % 2}":
                               outproj_unit(0, mt, cs, tg, nc.vector)]))
        units.sort(key=lambda u: u[0])

        def emit_units(step, limit=99):
            while units and units[0][0] <= step and limit > 0:
                _, pieces = units.pop(0)
                for em in pieces:
                    em()
                limit -= 1

        pairs = [(qc, hp) for qc in range(NQC) for hp in range(NJT)]
        step = 0
        for pi, (qc, hp) in enumerate(pairs):
            # pair window: fillers due now run in the ot ring before this
            # pair's attnv accumulators are allocated
            emit_units(step + (NLT if pi == 0 else 4))
            ots = {h: psum.tile([HD + 1, QC], F32, tag=f"ot{h}",
                                name=f"ot{h}")
                   for h in (0, 1)}
            prev = None
            for kt in range(NLT):
                split0 = step == 0
                at0 = att_pool.tile([128, QC], BF16, tag="at0", name="at0")
                at1 = att_pool.tile([128, QC], BF16, tag="at1", name="at1")
                if split0:
                    # first step: per-cs exps so ACT starts on the first
                    # q-chunk before xq c1 has even arrived
                    sta = sc_h(hp, qc, kt, 0, "stA", True)
                    stb = sc_h(hp, qc, kt, 1, "stB", True)
                    _exp(at0[:, 0:CS], sta[0])
                    _exp(at1[:, 0:CS], stb[0])
                    _exp(at0[:, CS:QC], sta[1])
                    _exp(at1[:, CS:QC], stb[1])
                else:
                    sta = sc_h(hp, qc, kt, 0, "stA")
                    _exp(at0, sta)
                    stb = sc_h(hp, qc, kt, 1, "stB")
                    _exp(at1, stb)
                if prev is not None:
                    attnv(hp, kt - 1, 0, prev[0], ots[0])
                    attnv(hp, kt - 1, 1, prev[1], ots[1])
                prev = (at0, at1)
                step += 1
            attnv(hp, NLT - 1, 0, prev[0], ots[0])
            attnv(hp, NLT - 1, 1, prev[1], ots[1])
            if pi == 3:
                # reserved out-proj(qc0) units bridge the PE gap over the
                # final epilogue; stA/stB are free once the last exps read
                for i, (mt, cs) in enumerate(((5, 0), (5, 1), (6, 0),
                                              (6, 1), (7, 0), (7, 1))):
                    outproj_unit(0, mt, cs, ("stA", "stB")[i % 2],
                                 nc.vector)
            epilogue(hp, qc, ots, chunks=2)
        emit_units(99)
        # tail: out-proj of the last q-chunk.  cs0 units first (gated only
        # on the final epilogue's first 512-wide chunk); copies split
        # between the now-idle ACT and DVE.
        ti = 0
        for c in range(NCS):
            for mt in range(NDT):
                outproj_unit(1, mt, c,
                             ("stA", "stB", "ot0", "ot1")[ti % 4],
                             (nc.scalar, nc.vector)[ti % 2])
                ti += 1

    nc.compile()
    return nc


_NC_CACHE = {}


def _get_nc():
    if "nc" not in _NC_CACHE:
        _NC_CACHE["nc"] = build_attention_core()
    return _NC_CACHE["nc"]


def _pack_x(x):
    """[L, D] fp32 -> [128, NDT, L] bf16 with d = t*128 + p."""
    xT = np.ascontiguousarray(np.asarray(x, np.float32).T)       # [D, L]
    return np.ascontiguousarray(
        xT.reshape(8, 128, xT.shape[1]).transpose(1, 0, 2)).astype(NPBF16)


def _pack_w(w):
    """[D, JC] fp32 -> [128, NDT, JC] bf16."""
    w = np.asarray(w, np.float32)
    return np.ascontiguousarray(
        w.reshape(8, 128, w.shape[1])).transpose(1, 0, 2).astype(NPBF16)


def shard_inputs(query, key_, value, Wq, bq, Wk, bk, Wv, bv, Wo, bo,
                 B=2, H=16, H_LOC=4, HD=64):
    """Host-side sharding: core c -> (batch c//4, head-group c%4)."""
    groups = H // H_LOC
    JC = H_LOC * HD
    xq = [_pack_x(np.asarray(query, np.float32)[b]) for b in range(B)]
    xk = [_pack_x(np.asarray(key_, np.float32)[b]) for b in range(B)]
    xv = [_pack_x(np.asarray(value, np.float32)[b]) for b in range(B)]
    in_maps = []
    for c in range(B * groups):
        b, g = divmod(c, groups)
        js = slice(g * JC, (g + 1) * JC)
        wo_s = np.asarray(Wo, np.float32)[js, :]                  # [JC, D]
        in_maps.append({
            "xq": xq[b], "xk": xk[b], "xv": xv[b],
            "wq": _pack_w(np.asarray(Wq, np.float32)[:, js]),
            "wk": _pack_w(np.asarray(Wk, np.float32)[:, js]),
            "wv": _pack_w(np.asarray(Wv, np.float32)[:, js]),
            "wo": np.ascontiguousarray(
                wo_s.reshape(2, 128, -1)).transpose(1, 0, 2).astype(NPBF16),
            "bq": np.ascontiguousarray(
                np.asarray(bq, np.float32)[js].reshape(2, 128).T),
            "bk": np.ascontiguousarray(
                np.asarray(bk, np.float32)[js].reshape(2, 128).T),
            "bv": np.ascontiguousarray(np.broadcast_to(
                np.asarray(bv, np.float32)[js], (128, JC))),
        })
    return in_maps


def kernel(query, key_, value, Wq, bq, Wk, bk, Wv, bv, Wo, bo):
    B, L, D = 2, 2048, 1024
    groups = 4
    nc = _get_nc()
    in_maps = shard_inputs(query, key_, value, Wq, bq, Wk, bk, Wv, bv, Wo, bo)
    res = run_bass_kernel_spmd(nc, in_maps, list(range(8))).results
    out = np.empty((B, L, D), np.float32)
    bo = np.asarray(bo, np.float32)
    for b in range(B):
        acc = res[b * groups]["outT"].astype(np.float32)
        for g in range(1, groups):
            acc = acc + res[b * groups + g]["outT"]
        out[b] = acc.T + bo
    return out


# revision 47
# speedup vs baseline: 1.0018x; 1.0018x over previous
"""Memory-efficient multi-head attention on 8 TRN2 NeuronCores (bf16 core).

Problem (hardcoded): B=2, L=2048, D=1024, H=16, HD=64.
  out = softmax((X_q Wq + bq)(X_k Wk + bk)^T / sqrt(HD)) (X_v Wv + bv) Wo + bo

Sharding: 8 cores = 2 batches x 4 head-groups (4 heads each).  Each core gets
its batch's activations (transposed, bf16) plus its head-group's weight
slices, and produces a partial pre-bias output outT [D, L] in fp32.  Host
sums the 4 partials per batch and adds bo.

Device kernel design (cost-model-driven):
  - All matmul operands are bf16 (1 PE cycle/row, same as fp32r, but half
    the HBM traffic and SBUF).  PSUM accumulation stays fp32.  bf16
    per-element rounding (~0.1% rms) contributes ~0.2% output error vs the
    2e-2 budget.  (fp8 DoubleRow was evaluated and measured ~9% error - the
    attention average shrinks signal as fast as noise, so per-element fp8
    error passes through at full strength.  Rejected.)
  - Scores are computed transposed, sT[kpos, q] = k . q, per head on disjoint
    64-partition groups; softmax denominator comes from an appended ones
    column on v (attnv accumulator row 64).
  - exp on ScalarE with the 1/sqrt(HD) scale folded in, no max-subtraction
    (scores ~N(0,1); softmax is shift-invariant; fp32/bf16 range is ample).
  - Biases are applied by DVE during PSUM->SBUF copy-out (tensor_scalar_add
    with per-partition bias for q/k, tensor_tensor add with a host-
    broadcast bias tile for v) - no PE bias matmuls.
  - A Pool-engine memset chain paces tiny dummy matmuls over the first
    ~6us so the PE p-state ramp (cost model: warm after 3us of continuous
    activity; resets only after a >3us idle gap) is done before real
    matmuls arrive - every real matmul runs at the warm 2.4GHz rate.
  - PE is the bottleneck (163.7us busy, its exact cost-model floor).
    PSUM banks: stA/stB [128,1024] score slots (the two heads' exps
    alternate, so each head's next score lands while the other head's exp
    runs - zero-stall ACT cadence) + ot0/ot1 [65,1024] attnv accumulators.
    All projection and out-proj(qc0) "filler" work runs through the ot
    slot ring: before each pair's first attnv (deferred via the 10-deep
    at-tile pool) and at pair boundaries, with deadlines matched to the
    single-queue DMA arrival order so nothing parks in the 4-deep PE wait
    queue.  out-proj(qc1) is the tail, cs-ordered behind the chunked
    final epilogue, with copies split between the then-idle ACT and DVE
    and bf16 stores to halve the final DMA.
"""

import os
import numpy as np
import ml_dtypes

EMIT_LOG = []  # (inst_name, label) when KERNEL_EMIT_LOG=1

import concourse.mybir as mybir
import concourse.tile as tile
from concourse import bacc
from concourse.bass_utils import run_bass_kernel_spmd

F32 = mybir.dt.float32
BF16 = mybir.dt.bfloat16
NPBF16 = ml_dtypes.bfloat16


def build_attention_core(L=2048, D=1024, H_LOC=4, HD=64):
    JC = H_LOC * HD                   # local head channels (256)
    NJT = JC // 128                   # j-tiles (2)
    NDT = D // 128                    # contraction tiles (8)
    NLT = L // 128                    # k-position tiles (16)
    XC = 512                          # x stream chunk width
    NXC = L // XC                     # 4
    QC = 1024                         # attention q-chunk (exp tile width)
    NQC = L // QC                     # 2
    CS = 512                          # matmul N-chunk inside a q-chunk
    NCS = QC // CS                    # 2

    nc = bacc.Bacc("TRN2", target_bir_lowering=False, debug=False, num_devices=8)

    xq = nc.dram_tensor("xq", [128, NDT, L], BF16, kind="ExternalInput").ap()
    xk = nc.dram_tensor("xk", [128, NDT, L], BF16, kind="ExternalInput").ap()
    xv = nc.dram_tensor("xv", [128, NDT, L], BF16, kind="ExternalInput").ap()
    wq = nc.dram_tensor("wq", [128, NDT, JC], BF16, kind="ExternalInput").ap()
    wk = nc.dram_tensor("wk", [128, NDT, JC], BF16, kind="ExternalInput").ap()
    wv = nc.dram_tensor("wv", [128, NDT, JC], BF16, kind="ExternalInput").ap()
    wo = nc.dram_tensor("wo", [128, NJT, D], BF16, kind="ExternalInput").ap()
    bq = nc.dram_tensor("bq", [128, NJT], F32, kind="ExternalInput").ap()
    bk = nc.dram_tensor("bk", [128, NJT], F32, kind="ExternalInput").ap()
    bv = nc.dram_tensor("bv", [128, JC], F32, kind="ExternalInput").ap()
    outT = nc.dram_tensor("outT", [D, L], BF16, kind="ExternalOutput").ap()

    from contextlib import ExitStack

    with (
        tile.TileContext(nc) as tc,
        ExitStack() as stack,
        nc.allow_low_precision(reason="bf16 operands, fp32 accumulation"),
    ):
        consts = stack.enter_context(tc.tile_pool(name="consts", bufs=1))
        warm = stack.enter_context(tc.tile_pool(name="warm", bufs=1))
        wpool = stack.enter_context(tc.tile_pool(name="wpool", bufs=1))
        xpool = stack.enter_context(tc.tile_pool(name="xpool", bufs=1))
        prod = stack.enter_context(tc.tile_pool(name="prod", bufs=1))
        att_pool = stack.enter_context(tc.tile_pool(name="att", bufs=10))
        rden_pool = stack.enter_context(tc.tile_pool(name="rden", bufs=2))
        oc_pool = stack.enter_context(tc.tile_pool(name="oc", bufs=8))
        psum = stack.enter_context(tc.tile_pool(name="psum", bufs=1, space="PSUM"))

        # ---- PE warmup: Pool memset chain paces tiny matmuls ~1.5us apart
        # (prevents a pe_busy_start reset during the DMA prefix; each reset
        # makes the next ~3us of matmuls run at the cold p-state)
        wsrc = warm.tile([1, 8], BF16, tag="wsrc")
        nc.vector.memset(wsrc, 0.0)
        ps_wm = psum.tile([1, 8], F32, tag="stA", name="ps_wm")
        for i in range(2):
            wch = warm.tile([1, 1500], F32, tag="wch", name="wch")
            nc.gpsimd.memset(wch, 0.0)
            gate = warm.tile([1, 2], BF16, tag="wg", bufs=2, name="gate")
            nc.vector.tensor_copy(out=gate, in_=wch[:, 0:2])
            nc.tensor.matmul(ps_wm[:, 0:1], gate[:, 0:1], gate[:, 1:2],
                             start=True, stop=True)

        # ---- weights + biases (lead the DMA queue)
        wk_sb = wpool.tile([128, NDT, JC], BF16, tag="wk")
        wq_sb = wpool.tile([128, NDT, JC], BF16, tag="wq")
        wv_sb = wpool.tile([128, NDT, JC], BF16, tag="wv")
        wo_sb = wpool.tile([128, NJT, D], BF16, tag="wo")
        bq_sb = wpool.tile([128, NJT], F32, tag="bq")
        bk_sb = wpool.tile([128, NJT], F32, tag="bk")
        bv_sb = wpool.tile([128, JC], F32, tag="bv")
        nc.sync.dma_start(out=wk_sb[:, 0:4, :], in_=wk[:, 0:4, :])

        # ---- x streams (SBUF-resident, chunked DMA)
        xk_sb = xpool.tile([128, NDT, L], BF16, tag="xk")
        xq_sb = xpool.tile([128, NDT, L], BF16, tag="xq")
        xv_sb = xpool.tile([128, NDT, L], BF16, tag="xv")

        def load_x(eng, x_sb, x_dram, c):
            eng.dma_start(out=x_sb[:, :, c * XC:(c + 1) * XC],
                          in_=x_dram[:, :, c * XC:(c + 1) * XC])

        # All loads on ONE queue (SP) in priority order: the DMA_ENGINES
        # device serializes transfers, so a second queue would let late
        # loads jump ahead of the critical prefix.
        nc.sync.dma_start(out=xk_sb[:, 0:4, 0:XC], in_=xk[:, 0:4, 0:XC])
        nc.sync.dma_start(out=bk_sb, in_=bk)
        nc.sync.dma_start(out=wk_sb[:, 4:8, :], in_=wk[:, 4:8, :])
        nc.sync.dma_start(out=xk_sb[:, 4:8, 0:XC], in_=xk[:, 4:8, 0:XC])
        nc.sync.dma_start(out=wq_sb[:, 0:4, :], in_=wq[:, 0:4, :])
        nc.sync.dma_start(out=xq_sb[:, 0:4, 0:XC], in_=xq[:, 0:4, 0:XC])
        nc.sync.dma_start(out=bq_sb, in_=bq)
        nc.sync.dma_start(out=wq_sb[:, 4:8, :], in_=wq[:, 4:8, :])
        nc.sync.dma_start(out=xq_sb[:, 4:8, 0:XC], in_=xq[:, 4:8, 0:XC])
        load_x(nc.sync, xq_sb, xq, 1)
        nc.sync.dma_start(out=wv_sb, in_=wv)
        nc.sync.dma_start(out=bv_sb, in_=bv)
        load_x(nc.sync, xk_sb, xk, 1)
        load_x(nc.sync, xv_sb, xv, 0)
        load_x(nc.sync, xk_sb, xk, 2)
        load_x(nc.sync, xv_sb, xv, 1)
        load_x(nc.sync, xk_sb, xk, 3)
        load_x(nc.sync, xv_sb, xv, 2)
        load_x(nc.sync, xv_sb, xv, 3)
        load_x(nc.sync, xq_sb, xq, 2)
        load_x(nc.sync, xq_sb, xq, 3)
        nc.sync.dma_start(out=wo_sb, in_=wo)

        # ---- products
        qT_sb = prod.tile([128, NJT, L], BF16, tag="qT")    # [j%128, jt, l]
        kT_sb = prod.tile([128, NJT, L], BF16, tag="kT")
        v_sb = prod.tile([128, NLT, H_LOC, HD + 1], BF16, tag="v")
        onorm_sb = prod.tile([128, NJT, L], BF16, tag="onorm")

        vones_f32 = consts.tile([128, NLT * H_LOC], F32)
        nc.vector.memset(vones_f32, 1.0)
        nc.vector.tensor_copy(
            out=v_sb[:, :, :, HD:HD + 1].rearrange("p a b c -> p (a b c)"),
            in_=vones_f32,
        )

        # ---- emission helpers ------------------------------------------
        # Projections are emitted as small "pieces" (2 contraction matmuls
        # each, ~430ns of PE) so that, interleaved into the attention
        # stream, they never delay the next score matmul in the in-order
        # PE queue by more than ~one piece.
        def proj_kq_pieces(w_sb, b_sb, dst, c, jt, tag="fl"):
            x_sb = xk_sb if dst is kT_sb else xq_sb
            state = {}

            def piece(p):
                def emit():
                    if p == 0:
                        state["ps"] = psum.tile([128, XC], F32, tag=tag,
                                                name="pskq")
                    _log(f"projkq c{c} jt{jt} p{p}")
                    ps = state["ps"]
                    for dt in (2 * p, 2 * p + 1):
                        nc.tensor.matmul(
                            ps,
                            w_sb[:, dt, jt * 128:(jt + 1) * 128],
                            x_sb[:, dt, c * XC:(c + 1) * XC],
                            start=(dt == 0),
                            stop=(dt == NDT - 1),
                        )
                    if p == NDT // 2 - 1:
                        nc.vector.tensor_scalar_add(
                            out=dst[:, jt, c * XC:(c + 1) * XC],
                            in0=ps,
                            scalar1=b_sb[:, jt:jt + 1],
                        )
                return emit
            return [piece(p) for p in range(NDT // 2)]

        def proj_v_pieces(lt, tag="fl"):
            state = {}

            def piece(p):
                def emit():
                    if p == 0:
                        state["ps"] = psum.tile([128, JC], F32, tag=tag,
                                                name="psv")
                    _log(f"projv lt{lt} p{p}")
                    ps = state["ps"]
                    for dt in (2 * p, 2 * p + 1):
                        nc.tensor.matmul(
                            ps,
                            xv_sb[:, dt, lt * 128:lt * 128 + 128],
                            wv_sb[:, dt, :],
                            start=(dt == 0),
                            stop=(dt == NDT - 1),
                        )
                    if p == NDT // 2 - 1:
                        nc.vector.tensor_tensor(
                            out=v_sb[:, lt, :, 0:HD],
                            in0=ps.rearrange("p (h d) -> p h d", h=H_LOC),
                            in1=bv_sb.rearrange("p (h d) -> p h d", h=H_LOC),
                            op=mybir.AluOpType.add,
                        )
                return emit
            return [piece(p) for p in range(NDT // 2)]

        EXP_SCALE = float(1.0 / np.sqrt(HD))

        _log_on = os.environ.get("KERNEL_EMIT_LOG") == "1"

        def _log(label):
            if _log_on:
                # peek: next add_instruction gets this id + 1 (next_id consumes one)
                nid = nc.next_id()
                EMIT_LOG.append((f"I-{nid + 1}", label))

        def _score_mm(ps, hb, hp, kt, q0, q1):
            _log(f"score hb{hb} hp{hp} kt{kt} q{q0}")
            nc.tensor.matmul(
                ps,
                kT_sb[hb:hb + HD, hp, kt * 128:kt * 128 + 128],
                qT_sb[hb:hb + HD, hp, q0:q1],
                start=True,
                stop=True,
            )

        def _exp(out, in_):
            _log("exp")
            nc.scalar.activation(out=out, in_=in_,
                                 func=mybir.ActivationFunctionType.Exp,
                                 scale=EXP_SCALE)

        def sc_h(hp, qc, kt, h, tag, split0=False):
            """One head's scores for a kt into its dedicated 2-bank slot."""
            q0 = qc * QC
            hb = h * HD
            if split0:
                tiles = []
                for c in range(NCS):
                    st = psum.tile([128, CS], F32, tag=tag, name=tag)
                    _score_mm(st, hb, hp, kt, q0 + c * CS, q0 + (c + 1) * CS)
                    tiles.append(st)
                return tiles
            st = psum.tile([128, QC], F32, tag=tag, name=tag)
            for c in range(NCS):
                _score_mm(st[:, c * CS:(c + 1) * CS], hb, hp, kt,
                          q0 + c * CS, q0 + (c + 1) * CS)
            return st

        def attnv(hp, kt, h, at, ot):
            hg = 2 * hp + (h % 2)
            for c in range(NCS):
                _log(f"attnv hp{hp} kt{kt} h{h}")
                nc.tensor.matmul(
                    ot[:, c * CS:(c + 1) * CS],
                    v_sb[:, kt, hg, :],
                    at[:, c * CS:(c + 1) * CS],
                    start=(kt == 0),
                    stop=(kt == NLT - 1),
                )

        def epilogue(hp, qc, ots, chunks=1):
            """onorm[ch, q] = OT[ch, q] * recip(OT[HD, q])."""
            W = QC // chunks
            for cc in range(chunks):
                for h in (0, 1):
                    hb = h * HD
                    rden = rden_pool.tile([1, W], F32, tag="rden", name="rden")
                    nc.vector.reciprocal(
                        out=rden, in_=ots[h][HD:HD + 1, cc * W:(cc + 1) * W])
                    rbc = rden_pool.tile([HD, W], F32, tag="rbc", name="rbc")
                    nc.gpsimd.partition_broadcast(rbc, rden)
                    nc.vector.tensor_mul(
                        out=onorm_sb[hb:hb + HD, hp,
                                     qc * QC + cc * W:qc * QC + (cc + 1) * W],
                        in0=ots[h][0:HD, cc * W:(cc + 1) * W],
                        in1=rbc,
                    )

        _store_rr = [0]

        def outproj_unit(qc, mt, c, tag, copy_eng):
            """One [128, CS] tile of outT[dp, l] = sum_j wo[j, dp] onorm[j, l]."""
            ps = psum.tile([128, CS], F32, tag=tag, name="psc")
            _log(f"outproj qc{qc} mt{mt} c{c}")
            for jt in range(NJT):
                nc.tensor.matmul(
                    ps,
                    wo_sb[:, jt, mt * 128:(mt + 1) * 128],
                    onorm_sb[:, jt, qc * QC + c * CS:qc * QC + (c + 1) * CS],
                    start=(jt == 0),
                    stop=(jt == NJT - 1),
                )
            ob = oc_pool.tile([128, CS], BF16, tag="oc", name="oc")
            if copy_eng is nc.scalar:
                nc.scalar.copy(out=ob, in_=ps)
            else:
                copy_eng.tensor_copy(out=ob, in_=ps)
            dma_eng = nc.sync  # SP queue is idle once loads finish
            dma_eng.dma_start(
                out=outT[mt * 128:(mt + 1) * 128,
                         qc * QC + c * CS:qc * QC + (c + 1) * CS],
                in_=ob,
            )

        # ---- emission schedule -----------------------------------------
        # Both heads' exps are [128, QC] through dedicated 2-bank slots
        # (stA/stB) - the zero-stall cadence: while exp(h0) runs, PE scores
        # h1 into stB, and vice versa.  attnv trails via the deep at pool
        # (12 bufs/head), so ALL projection/out-proj filler units run in
        # the ot-slot ring: before the pair's first attnv (window 0 covers
        # every projection with an early deadline) and at pair boundaries
        # after each epilogue.  ACT stalls where PE is the binding engine;
        # PE should never idle.
        for em in proj_kq_pieces(wk_sb, bk_sb, kT_sb, 0, 0, tag="stA"):
            em()
        for em in proj_kq_pieces(wq_sb, bq_sb, qT_sb, 0, 0, tag="stB"):
            em()

        # filler units, deadline = global step by which the unit must be
        # DONE; emitted into the ot ring at window points.  DMA arrivals
        # (one load queue) gate the earliest useful emission.
        units = []
        units.append((1, proj_kq_pieces(wq_sb, bq_sb, qT_sb, 1, 0, tag="ot0")))
        units.append((3, proj_kq_pieces(wk_sb, bk_sb, kT_sb, 1, 0, tag="ot0")))
        units.append((2, proj_v_pieces(0, tag="ot1")))
        units.append((2, proj_v_pieces(1, tag="ot0")))
        units.append((3, proj_v_pieces(2, tag="ot1")))
        units.append((4, proj_v_pieces(3, tag="ot0")))
        units.append((5, proj_v_pieces(4, tag="ot1")))
        units.append((5, proj_kq_pieces(wk_sb, bk_sb, kT_sb, 2, 0, tag="ot0")))
        units.append((6, proj_v_pieces(5, tag="ot1")))
        units.append((0, proj_kq_pieces(wk_sb, bk_sb, kT_sb, 0, 1, tag="ot0")))
        units.append((7, proj_v_pieces(6, tag="ot1")))
        units.append((7, proj_kq_pieces(wq_sb, bq_sb, qT_sb, 0, 1, tag="ot0")))
        units.append((8, proj_v_pieces(7, tag="ot1")))
        units.append((8, proj_kq_pieces(wk_sb, bk_sb, kT_sb, 3, 0, tag="ot0")))
        units.append((9, proj_v_pieces(8, tag="ot1")))
        units.append((9, proj_kq_pieces(wq_sb, bq_sb, qT_sb, 1, 1, tag="ot0")))
        units.append((10, proj_v_pieces(9, tag="ot1")))
        units.append((10, proj_kq_pieces(wk_sb, bk_sb, kT_sb, 1, 1, tag="ot0")))
        units.append((11, proj_v_pieces(10, tag="ot1")))
        units.append((11, proj_v_pieces(11, tag="ot0")))
        units.append((12, proj_v_pieces(12, tag="ot1")))
        units.append((12, proj_kq_pieces(wk_sb, bk_sb, kT_sb, 2, 1, tag="ot0")))
        units.append((13, proj_v_pieces(13, tag="ot1")))
        units.append((13, proj_kq_pieces(wk_sb, bk_sb, kT_sb, 3, 1, tag="ot0")))
        units.append((14, proj_v_pieces(14, tag="ot1")))
        units.append((14, proj_v_pieces(15, tag="ot1")))
        units.append((18, proj_kq_pieces(wq_sb, bq_sb, qT_sb, 2, 0, tag="ot0")))
        units.append((19, proj_kq_pieces(wq_sb, bq_sb, qT_sb, 3, 0, tag="ot1")))
        units.append((33, proj_kq_pieces(wq_sb, bq_sb, qT_sb, 2, 1, tag="ot0")))
        units.append((34, proj_kq_pieces(wq_sb, bq_sb, qT_sb, 3, 1, tag="ot1")))
        for mt in range(NDT - 2):
            for cs in range(NCS):
                units.append((35 + (2 * mt + cs) % 10,
                              [lambda mt=mt, cs=cs, tg=f"ot{(2*mt+cs) % 2}":
                               outproj_unit(0, mt, cs, tg, nc.vector)]))
        units.sort(key=lambda u: u[0])

        def emit_units(step, limit=99):
            while units and units[0][0] <= step and limit > 0:
                _, pieces = units.pop(0)
                for em in pieces:
                    em()
                limit -= 1

        pairs = [(qc, hp) for qc in range(NQC) for hp in range(NJT)]
        step = 0
        for pi, (qc, hp) in enumerate(pairs):
            # pair window: fillers due now run in the ot ring before this
            # pair's attnv accumulators are allocated
            emit_units(step + (NLT if pi == 0 else 4))
            ots = {h: psum.tile([HD + 1, QC], F32, tag=f"ot{h}",
                                name=f"ot{h}")
                   for h in (0, 1)}
            prev = None
            for kt in range(NLT):
                split0 = step == 0
                at0 = att_pool.tile([128, QC], BF16, tag="at0", name="at0")
                at1 = att_pool.tile([128, QC], BF16, tag="at1", name="at1")
                if split0:
                    # first step: per-cs exps so ACT starts on the first
                    # q-chunk before xq c1 has even arrived
                    sta = sc_h(hp, qc, kt, 0, "stA", True)
                    stb = sc_h(hp, qc, kt, 1, "stB", True)
                    _exp(at0[:, 0:CS], sta[0])
                    _exp(at1[:, 0:CS], stb[0])
                    _exp(at0[:, CS:QC], sta[1])
                    _exp(at1[:, CS:QC], stb[1])
                else:
                    sta = sc_h(hp, qc, kt, 0, "stA")
                    _exp(at0, sta)
                    stb = sc_h(hp, qc, kt, 1, "stB")
                    _exp(at1, stb)
                if prev is not None:
                    attnv(hp, kt - 1, 0, prev[0], ots[0])
                    attnv(hp, kt - 1, 1, prev[1], ots[1])
                prev = (at0, at1)
                step += 1
            attnv(hp, NLT - 1, 0, prev[0], ots[0])
            attnv(hp, NLT - 1, 1, prev[1], ots[1])
            if pi == 3:
                # reserved out-proj(qc0) units bridge the PE gap over the
                # final epilogue; stA/stB are free once the last exps read
                for i, (mt, cs) in enumerate(((6, 0), (6, 1), (7, 0), (7, 1))):
                    outproj_unit(0, mt, cs, ("stA", "stB")[i % 2],
                                 nc.vector)
            epilogue(hp, qc, ots, chunks=2)
        emit_units(99)
        # tail: out-proj of the last q-chunk.  cs0 units first (gated only
        # on the final epilogue's first 512-wide chunk); copies split
        # between the now-idle ACT and DVE.
        ti = 0
        for c in range(NCS):
            for mt in range(NDT):
                outproj_unit(1, mt, c,
                             ("stA", "stB", "ot0", "ot1")[ti % 4],
                             (nc.scalar, nc.vector)[ti % 2])
                ti += 1

    nc.compile()
    return nc


_NC_CACHE = {}


def _get_nc():
    if "nc" not in _NC_CACHE:
        _NC_CACHE["nc"] = build_attention_core()
    return _NC_CACHE["nc"]


def _pack_x(x):
    """[L, D] fp32 -> [128, NDT, L] bf16 with d = t*128 + p."""
    xT = np.ascontiguousarray(np.asarray(x, np.float32).T)       # [D, L]
    return np.ascontiguousarray(
        xT.reshape(8, 128, xT.shape[1]).transpose(1, 0, 2)).astype(NPBF16)


def _pack_w(w):
    """[D, JC] fp32 -> [128, NDT, JC] bf16."""
    w = np.asarray(w, np.float32)
    return np.ascontiguousarray(
        w.reshape(8, 128, w.shape[1])).transpose(1, 0, 2).astype(NPBF16)


def shard_inputs(query, key_, value, Wq, bq, Wk, bk, Wv, bv, Wo, bo,
                 B=2, H=16, H_LOC=4, HD=64):
    """Host-side sharding: core c -> (batch c//4, head-group c%4)."""
    groups = H // H_LOC
    JC = H_LOC * HD
    xq = [_pack_x(np.asarray(query, np.float32)[b]) for b in range(B)]
    xk = [_pack_x(np.asarray(key_, np.float32)[b]) for b in range(B)]
    xv = [_pack_x(np.asarray(value, np.float32)[b]) for b in range(B)]
    in_maps = []
    for c in range(B * groups):
        b, g = divmod(c, groups)
        js = slice(g * JC, (g + 1) * JC)
        wo_s = np.asarray(Wo, np.float32)[js, :]                  # [JC, D]
        in_maps.append({
            "xq": xq[b], "xk": xk[b], "xv": xv[b],
            "wq": _pack_w(np.asarray(Wq, np.float32)[:, js]),
            "wk": _pack_w(np.asarray(Wk, np.float32)[:, js]),
            "wv": _pack_w(np.asarray(Wv, np.float32)[:, js]),
            "wo": np.ascontiguousarray(
                wo_s.reshape(2, 128, -1)).transpose(1, 0, 2).astype(NPBF16),
            "bq": np.ascontiguousarray(
                np.asarray(bq, np.float32)[js].reshape(2, 128).T),
            "bk": np.ascontiguousarray(
                np.asarray(bk, np.float32)[js].reshape(2, 128).T),
            "bv": np.ascontiguousarray(np.broadcast_to(
                np.asarray(bv, np.float32)[js], (128, JC))),
        })
    return in_maps


def kernel(query, key_, value, Wq, bq, Wk, bk, Wv, bv, Wo, bo):
    B, L, D = 2, 2048, 1024
    groups = 4
    nc = _get_nc()
    in_maps = shard_inputs(query, key_, value, Wq, bq, Wk, bk, Wv, bv, Wo, bo)
    res = run_bass_kernel_spmd(nc, in_maps, list(range(8))).results
    out = np.empty((B, L, D), np.float32)
    bo = np.asarray(bo, np.float32)
    for b in range(B):
        acc = res[b * groups]["outT"].astype(np.float32)
        for g in range(1, groups):
            acc = acc + res[b * groups + g]["outT"]
        out[b] = acc.T + bo
    return out
